# revision 6
# baseline (speedup 1.0000x reference)
"""ChannelAttention (CBAM-style) Trainium2 Bass kernel.

Reference computation (per batch image):
    avg = mean(x, spatial)             # [C]
    mx  = max(x, spatial)              # [C]
    s   = sigmoid(mlp(avg) + mlp(max)) # mlp: relu(p@w1+b1)@w2+b2
    y   = x * s[None, None, :]

Full shapes: x [32, 112, 112, 256] f32, w1 [256, 32], b1 [32], w2 [32, 256],
b2 [256].  Data-parallel over batch: each of the 8 NeuronCores handles 4
images; the tiny MLP weights are replicated.

Per-core layout: x viewed as [4, 12544, 256] rows.  Each image is split
into 128 partitions x 98 spatial rows ("(p t) c" with p=128, t=98), and
processed in 7 chunks of 14 spatial rows so DMA (fully contiguous 14336B
per partition per chunk) overlaps compute:
  - sum-pool: PE matmul with an all-ones stationary column accumulating
    [1, 256] channel sums in PSUM across all 98 row-tiles.
  - max-pool: ACT-engine tensor_tensor(max) accumulation into a
    [128, 7, 256] accumulator, folded 7->1 on DVE, then PE-transposed to
    channel-major and reduce_max'd to [128, 1] per 128-channel block.
  - MLP runs on PE/ACT at [K<=128, N<=256] sizes; sigmoid on ACT; the
    [1, 256] scale row is broadcast to [128, 256] via a K=1 outer-product
    matmul with ones.
  - scale: DVE tensor_tensor(mult) with a free-dim-broadcast AP, in place
    on the resident chunk, then stored back (ACT-issued HWDGE ring so
    loads on the SP ring proceed independently).
"""

import sys

import numpy as np

for _p in ("/opt/trn_rl_repo",):
    if _p not in sys.path:
        sys.path.append(_p)

import concourse.bass as bass
import concourse.tile as tile
from concourse import mybir

B, HW, C = 32, 112 * 112, 256  # 12544 spatial positions per image
HID = 32
N_CORES = 8
IMG_PER_CORE = B // N_CORES  # 4
P = 128
T = HW // P  # 98 spatial rows per partition
CHUNK_T = 14  # rows per chunk
N_CHUNK = T // CHUNK_T  # 7
F32 = mybir.dt.float32


def _split_multiwait(nc, max_waits=1):
    """This walrus build rejects >1 sync wait per instruction.  Hoist
    excess waits onto InstNoOp instructions inserted just before, on the
    same engine (same semantics: the sequencer blocks on each in turn)."""
    for f in nc.m.functions:
        for bb in f.blocks:
            new_insts = []
            for ins in bb.instructions:
                si = ins.sync_info
                w = list(si.on_wait) if si and si.on_wait else []
                if len(w) > max_waits:
                    for j, ww in enumerate(w[:-max_waits]):
                        nop = mybir.InstNoOp(
                            name=f"{ins.name}.sw{j}",
                            engine=ins.engine,
                            sync_info=mybir.SyncInfo(on_wait=[ww], on_update=[]),
                        )
                        nc.register_instruction(nop, overwrite=True)
                        new_insts.append(nop)
                    si.on_wait = w[-max_waits:]
                new_insts.append(ins)
            bb.instructions = new_insts


def build_nc(n_img=IMG_PER_CORE, chunk_bufs=9, repeat=1):
    nc = bass.Bass()
    rows = n_img * HW
    x_d = nc.declare_dram_parameter("x", [rows, C], F32, isOutput=False)
    w1_d = nc.declare_dram_parameter("w1", [C, HID], F32, isOutput=False)
    b1_d = nc.declare_dram_parameter("b1", [HID], F32, isOutput=False)
    w2_d = nc.declare_dram_parameter("w2", [HID, C], F32, isOutput=False)
    b2_d = nc.declare_dram_parameter("b2", [C], F32, isOutput=False)
    id_d = nc.declare_dram_parameter("ident", [P, P], F32, isOutput=False)
    y_d = nc.declare_dram_parameter("y", [rows, C], F32, isOutput=True)

    xv = x_d.rearrange("(i p t) c -> i p t c", i=n_img, p=P)
    yv = y_d.rearrange("(i p t) c -> i p t c", i=n_img, p=P)

    AF = mybir.ActivationFunctionType
    OP = mybir.AluOpType

    with tile.TileContext(nc) as tc:
        with (
            tc.tile_pool(name="singles", bufs=1) as singles,
            tc.tile_pool(name="chunks", bufs=chunk_bufs) as chunks_pool,
            tc.tile_pool(name="maxacc", bufs=2) as maxacc_pool,
            tc.tile_pool(name="srep", bufs=2) as srep_pool,
            tc.tile_pool(name="small", bufs=3) as small,
            tc.tile_pool(name="ps_sum", bufs=2, space="PSUM") as ps_sum_pool,
            tc.tile_pool(name="ps_small", bufs=2, space="PSUM") as ps_small_pool,
            tc.tile_pool(name="ps_bc", bufs=2, space="PSUM") as ps_bc_pool,
        ):
            # --- constants ---
            w1_sb = singles.tile([P, 2, HID], F32)
            nc.sync.dma_start(out=w1_sb[:], in_=w1_d.rearrange("(b p) h -> p b h", p=P))
            w2_sb = singles.tile([HID, C], F32)
            nc.sync.dma_start(out=w2_sb[:], in_=w2_d[:, :])
            b1_sb = singles.tile([HID, 1], F32)
            nc.sync.dma_start(out=b1_sb[:], in_=b1_d.rearrange("(p o) -> p o", o=1))
            b2x2 = singles.tile([1, C], F32)
            nc.sync.dma_start(out=b2x2[:], in_=b2_d.rearrange("(o c) -> o c", o=1))
            nc.scalar.mul(out=b2x2[:], in_=b2x2[:], mul=2.0)
            ident = singles.tile([P, P], F32)
            nc.sync.dma_start(out=ident[:], in_=id_d[:, :])
            ones_col = singles.tile([P, 1], F32)
            nc.vector.memset(ones_col[:], 1.0)
            ones_row = singles.tile([1, P], F32)
            nc.vector.memset(ones_row[:], 1.0)

            for img in [i for _ in range(repeat) for i in range(n_img)]:
                psum_sum = ps_sum_pool.tile([1, C], F32)
                maxacc = maxacc_pool.tile([P, N_CHUNK, C], F32)
                cks = []
                for g in range(N_CHUNK):
                    ck = chunks_pool.tile([P, CHUNK_T, C], F32, tag="ck")
                    cks.append(ck)
                    nc.sync.dma_start(
                        out=ck[:], in_=xv[img][:, g * CHUNK_T : (g + 1) * CHUNK_T, :]
                    )
                    # sum-pool: accumulate channel sums in PSUM
                    for t in range(CHUNK_T):
                        nc.tensor.matmul(
                            psum_sum[:],
                            lhsT=ones_col[:],
                            rhs=ck[:, t, :],
                            start=(g == 0 and t == 0),
                            stop=(g == N_CHUNK - 1 and t == CHUNK_T - 1),
                        )
                    # max-pool accumulate on ACT (keeps DVE free for scaling)
                    if g == 0:
                        nc.any.tensor_copy(out=maxacc[:], in_=ck[:, 0:N_CHUNK, :])
                    else:
                        nc.any.tensor_tensor(
                            out=maxacc[:],
                            in0=maxacc[:],
                            in1=ck[:, 0:N_CHUNK, :],
                            op=OP.max,
                        )
                    nc.any.tensor_tensor(
                        out=maxacc[:],
                        in0=maxacc[:],
                        in1=ck[:, N_CHUNK : 2 * N_CHUNK, :],
                        op=OP.max,
                    )

                # --- pooled vectors ---
                # avg: psum row -> sbuf row (scaled by 1/HW)
                avg_row = small.tile([1, C], F32, tag="avg_row")
                nc.scalar.activation(
                    out=avg_row[:], in_=psum_sum[:], func=AF.Copy, scale=1.0 / HW
                )
                # max: fold 7 -> 1 in place on DVE
                m = maxacc
                nc.vector.tensor_tensor(m[:, 0, :], m[:, 0, :], m[:, 1, :], op=OP.max)
                nc.vector.tensor_tensor(m[:, 2, :], m[:, 2, :], m[:, 3, :], op=OP.max)
                nc.vector.tensor_tensor(m[:, 4, :], m[:, 4, :], m[:, 5, :], op=OP.max)
                nc.vector.tensor_tensor(m[:, 0, :], m[:, 0, :], m[:, 2, :], op=OP.max)
                nc.vector.tensor_tensor(m[:, 4, :], m[:, 4, :], m[:, 6, :], op=OP.max)
                nc.vector.tensor_tensor(m[:, 0, :], m[:, 0, :], m[:, 4, :], op=OP.max)

                # pooled columns: [128, 4] = avg_b0, avg_b1, max_b0, max_b1
                pooled = small.tile([P, 4], F32, tag="pooled")
                for blk in range(2):
                    pc = ps_small_pool.tile([P, 1], F32, tag="ps")
                    nc.tensor.matmul(
                        pc[:],
                        lhsT=avg_row[:, blk * P : (blk + 1) * P],
                        rhs=ones_col[0:1, 0:1],
                        start=True,
                        stop=True,
                    )
                    nc.any.tensor_copy(out=pooled[:, blk : blk + 1], in_=pc[:])
                for blk in range(2):
                    pt = ps_small_pool.tile([P, P], F32, tag="ps")
                    nc.tensor.transpose(
                        pt[:], m[:, 0, blk * P : (blk + 1) * P], ident[:]
                    )
                    nc.vector.reduce_max(
                        out=pooled[:, 2 + blk : 3 + blk],
                        in_=pt[:],
                        axis=mybir.AxisListType.X,
                    )

                # --- shared MLP on both pooled vectors ---
                h_sb = small.tile([HID, 2], F32, tag="h")
                for j in range(2):  # 0: avg path, 1: max path
                    ph = ps_small_pool.tile([HID, 1], F32, tag="ps")
                    nc.tensor.matmul(
                        ph[:],
                        lhsT=w1_sb[:, 0, :],
                        rhs=pooled[:, 2 * j : 2 * j + 1],
                        start=True,
                        stop=False,
                    )
                    nc.tensor.matmul(
                        ph[:],
                        lhsT=w1_sb[:, 1, :],
                        rhs=pooled[:, 2 * j + 1 : 2 * j + 2],
                        start=False,
                        stop=True,
                    )
                    nc.scalar.activation(
                        out=h_sb[:, j : j + 1],
                        in_=ph[:],
                        func=AF.Relu,
                        bias=b1_sb[:],
                        scale=1.0,
                    )
                py = ps_small_pool.tile([1, C], F32, tag="ps")
                nc.tensor.matmul(
                    py[:], lhsT=h_sb[:, 0:1], rhs=w2_sb[:], start=True, stop=False
                )
                nc.tensor.matmul(
                    py[:], lhsT=h_sb[:, 1:2], rhs=w2_sb[:], start=False, stop=True
                )
                sig_row = small.tile([1, C], F32, tag="sig")
                nc.vector.tensor_add(out=sig_row[:], in0=py[:], in1=b2x2[:])
                nc.scalar.activation(out=sig_row[:], in_=sig_row[:], func=AF.Sigmoid)

                # broadcast scale row to all 128 partitions (outer product)
                pbc = ps_bc_pool.tile([P, C], F32)
                nc.tensor.matmul(
                    pbc[:], lhsT=ones_row[:], rhs=sig_row[:], start=True, stop=True
                )
                srep = srep_pool.tile([P, 2, C], F32)
                nc.any.tensor_copy(out=srep[:, 0, :], in_=pbc[:])
                nc.any.tensor_copy(out=srep[:, 1, :], in_=pbc[:])
                srep3 = srep[:, :, :].rearrange("p b c -> p (b c)").rearrange(
                    "p (o f) -> p o f", o=1
                )

                # --- scale in place and store ---
                for g in range(N_CHUNK):
                    ck = cks[g]
                    v3 = ck[:, :, :].rearrange("p (a two) c -> p a (two c)", two=2)
                    _, s_b = bass.broadcast_tensor_aps(v3, srep3)
                    nc.any.tensor_tensor(out=v3, in0=v3, in1=s_b, op=OP.mult)
                    nc.scalar.dma_start(
                        out=yv[img][:, g * CHUNK_T : (g + 1) * CHUNK_T, :], in_=ck[:]
                    )

    _split_multiwait(nc)
    return nc


# ---------------------------------------------------------------------------
# host-side driver
# ---------------------------------------------------------------------------

_CACHED = {}


def _get_nc():
    if "nc" not in _CACHED:
        _CACHED["nc"] = build_nc()
    return _CACHED["nc"]


def kernel(x, w1, b1, w2, b2):
    from concourse.bass_utils import run_bass_kernel_spmd

    x = np.ascontiguousarray(np.asarray(x), dtype=np.float32)
    assert x.shape == (B, 112, 112, C)
    xr = x.reshape(B, HW, C)
    ident = np.eye(P, dtype=np.float32)
    in_maps = []
    for c in range(N_CORES):
        shard = np.ascontiguousarray(
            xr[c * IMG_PER_CORE : (c + 1) * IMG_PER_CORE].reshape(
                IMG_PER_CORE * HW, C
            )
        )
        in_maps.append(
            {
                "x": shard,
                "w1": np.ascontiguousarray(w1, dtype=np.float32),
                "b1": np.ascontiguousarray(b1, dtype=np.float32),
                "w2": np.ascontiguousarray(w2, dtype=np.float32),
                "b2": np.ascontiguousarray(b2, dtype=np.float32),
                "ident": ident,
            }
        )
    nc = _get_nc()
    res = run_bass_kernel_spmd(nc, in_maps, list(range(N_CORES)))
    out = np.empty((B, HW, C), dtype=np.float32)
    for c in range(N_CORES):
        out[c * IMG_PER_CORE : (c + 1) * IMG_PER_CORE] = res.results[c]["y"].reshape(
            IMG_PER_CORE, HW, C
        )
    return out.reshape(B, 112, 112, C)


# revision 8
# speedup vs baseline: 1.0058x; 1.0058x over previous
"""ChannelAttention (CBAM-style) Trainium2 Bass kernel.

Reference computation (per batch image):
    avg = mean(x, spatial)             # [C]
    mx  = max(x, spatial)              # [C]
    s   = sigmoid(mlp(avg) + mlp(max)) # mlp: relu(p@w1+b1)@w2+b2
    y   = x * s[None, None, :]

Full shapes: x [32, 112, 112, 256] f32, w1 [256, 32], b1 [32], w2 [32, 256],
b2 [256].  Data-parallel over batch: each of the 8 NeuronCores handles 4
images; the tiny MLP weights are replicated.

Per-core layout: x viewed as [4, 12544, 256] rows.  Each image is split
into 128 partitions x 98 spatial rows ("(p t) c" with p=128, t=98), and
processed in 7 chunks of 14 spatial rows so DMA (fully contiguous 14336B
per partition per chunk) overlaps compute:
  - sum-pool: PE matmul with an all-ones stationary column accumulating
    [1, 256] channel sums in PSUM across all 98 row-tiles.
  - max-pool: ACT-engine tensor_tensor(max) accumulation into a
    [128, 7, 256] accumulator, folded 7->1 on DVE, then PE-transposed to
    channel-major and reduce_max'd to [128, 1] per 128-channel block.
  - MLP runs on PE/ACT at [K<=128, N<=256] sizes; sigmoid on ACT; the
    [1, 256] scale row is broadcast to [128, 256] via a K=1 outer-product
    matmul with ones.
  - scale: DVE tensor_tensor(mult) with a free-dim-broadcast AP, in place
    on the resident chunk, then stored back (ACT-issued HWDGE ring so
    loads on the SP ring proceed independently).
"""

import sys

import numpy as np

for _p in ("/opt/trn_rl_repo",):
    if _p not in sys.path:
        sys.path.append(_p)

import concourse.bass as bass
import concourse.tile as tile
from concourse import mybir

B, HW, C = 32, 112 * 112, 256  # 12544 spatial positions per image
HID = 32
N_CORES = 8
IMG_PER_CORE = B // N_CORES  # 4
P = 128
T = HW // P  # 98 spatial rows per partition
CHUNK_T = 14  # rows per chunk
N_CHUNK = T // CHUNK_T  # 7
MAXW = 7  # max-pool accumulator width (rows)
F32 = mybir.dt.float32


def _split_multiwait(nc, max_waits=1):
    """This walrus build rejects >1 sync wait per instruction.  Hoist
    excess waits onto InstNoOp instructions inserted just before, on the
    same engine (same semantics: the sequencer blocks on each in turn)."""
    for f in nc.m.functions:
        for bb in f.blocks:
            new_insts = []
            for ins in bb.instructions:
                si = ins.sync_info
                w = list(si.on_wait) if si and si.on_wait else []
                if len(w) > max_waits:
                    for j, ww in enumerate(w[:-max_waits]):
                        nop = mybir.InstNoOp(
                            name=f"{ins.name}.sw{j}",
                            engine=ins.engine,
                            sync_info=mybir.SyncInfo(on_wait=[ww], on_update=[]),
                        )
                        nc.register_instruction(nop, overwrite=True)
                        new_insts.append(nop)
                    si.on_wait = w[-max_waits:]
                new_insts.append(ins)
            bb.instructions = new_insts


def build_nc(n_img=IMG_PER_CORE, chunk_bufs=9, repeat=1, chunk_t=CHUNK_T,
             store_engine="scalar"):
    n_chunk = T // chunk_t
    assert chunk_t % MAXW == 0 or MAXW % chunk_t == 0
    nc = bass.Bass()
    rows = n_img * HW
    x_d = nc.declare_dram_parameter("x", [rows, C], F32, isOutput=False)
    w1_d = nc.declare_dram_parameter("w1", [C, HID], F32, isOutput=False)
    b1_d = nc.declare_dram_parameter("b1", [HID], F32, isOutput=False)
    w2_d = nc.declare_dram_parameter("w2", [HID, C], F32, isOutput=False)
    b2_d = nc.declare_dram_parameter("b2", [C], F32, isOutput=False)
    id_d = nc.declare_dram_parameter("ident", [P, P], F32, isOutput=False)
    y_d = nc.declare_dram_parameter("y", [rows, C], F32, isOutput=True)

    xv = x_d.rearrange("(i p t) c -> i p t c", i=n_img, p=P)
    yv = y_d.rearrange("(i p t) c -> i p t c", i=n_img, p=P)

    AF = mybir.ActivationFunctionType
    OP = mybir.AluOpType

    with tile.TileContext(nc) as tc:
        with (
            tc.tile_pool(name="singles", bufs=1) as singles,
            tc.tile_pool(name="chunks", bufs=chunk_bufs) as chunks_pool,
            tc.tile_pool(name="maxacc", bufs=2) as maxacc_pool,
            tc.tile_pool(name="srep", bufs=2) as srep_pool,
            tc.tile_pool(name="small", bufs=3) as small,
            tc.tile_pool(name="ps_sum", bufs=2, space="PSUM") as ps_sum_pool,
            tc.tile_pool(name="ps_small", bufs=2, space="PSUM") as ps_small_pool,
            tc.tile_pool(name="ps_bc", bufs=2, space="PSUM") as ps_bc_pool,
        ):
            # --- constants ---
            w1_sb = singles.tile([P, 2, HID], F32)
            nc.sync.dma_start(out=w1_sb[:], in_=w1_d.rearrange("(b p) h -> p b h", p=P))
            w2_sb = singles.tile([HID, C], F32)
            nc.sync.dma_start(out=w2_sb[:], in_=w2_d[:, :])
            b1_sb = singles.tile([HID, 1], F32)
            nc.sync.dma_start(out=b1_sb[:], in_=b1_d.rearrange("(p o) -> p o", o=1))
            b2x2 = singles.tile([1, C], F32)
            nc.sync.dma_start(out=b2x2[:], in_=b2_d.rearrange("(o c) -> o c", o=1))
            nc.scalar.mul(out=b2x2[:], in_=b2x2[:], mul=2.0)
            ident = singles.tile([P, P], F32)
            nc.sync.dma_start(out=ident[:], in_=id_d[:, :])
            ones_col = singles.tile([P, 1], F32)
            nc.vector.memset(ones_col[:], 1.0)
            ones_row = singles.tile([1, P], F32)
            nc.vector.memset(ones_row[:], 1.0)

            for img in [i for _ in range(repeat) for i in range(n_img)]:
                psum_sum = ps_sum_pool.tile([1, C], F32)
                w = min(MAXW, chunk_t)
                maxacc = maxacc_pool.tile([P, MAXW, C], F32)
                cks = []
                for g in range(n_chunk):
                    ck = chunks_pool.tile([P, chunk_t, C], F32, tag="ck")
                    cks.append(ck)
                    nc.sync.dma_start(
                        out=ck[:], in_=xv[img][:, g * chunk_t : (g + 1) * chunk_t, :]
                    )
                    # sum-pool: accumulate channel sums in PSUM
                    for t in range(chunk_t):
                        nc.tensor.matmul(
                            psum_sum[:],
                            lhsT=ones_col[:],
                            rhs=ck[:, t, :],
                            start=(g == 0 and t == 0),
                            stop=(g == n_chunk - 1 and t == chunk_t - 1),
                        )
                    # max-pool accumulate in MAXW-row slices (Tile routes the
                    # tensor_tensor ops between DVE and ACT)
                    for s in range(chunk_t // w):
                        sl = ck[:, s * w : (s + 1) * w, :]
                        if g == 0 and s == 0:
                            nc.any.tensor_copy(out=maxacc[:, 0:w, :], in_=sl)
                        else:
                            nc.any.tensor_tensor(
                                out=maxacc[:, 0:w, :],
                                in0=maxacc[:, 0:w, :],
                                in1=sl,
                                op=OP.max,
                            )

                # --- pooled vectors ---
                # avg: psum row -> sbuf row (scaled by 1/HW)
                avg_row = small.tile([1, C], F32, tag="avg_row")
                nc.scalar.activation(
                    out=avg_row[:], in_=psum_sum[:], func=AF.Copy, scale=1.0 / HW
                )
                # max: fold MAXW -> 1 in place on DVE
                m = maxacc
                nc.vector.tensor_tensor(m[:, 0, :], m[:, 0, :], m[:, 1, :], op=OP.max)
                nc.vector.tensor_tensor(m[:, 2, :], m[:, 2, :], m[:, 3, :], op=OP.max)
                nc.vector.tensor_tensor(m[:, 4, :], m[:, 4, :], m[:, 5, :], op=OP.max)
                nc.vector.tensor_tensor(m[:, 0, :], m[:, 0, :], m[:, 2, :], op=OP.max)
                nc.vector.tensor_tensor(m[:, 4, :], m[:, 4, :], m[:, 6, :], op=OP.max)
                nc.vector.tensor_tensor(m[:, 0, :], m[:, 0, :], m[:, 4, :], op=OP.max)

                # pooled columns: [128, 4] = avg_b0, avg_b1, max_b0, max_b1
                pooled = small.tile([P, 4], F32, tag="pooled")
                for blk in range(2):
                    pc = ps_small_pool.tile([P, 1], F32, tag="ps")
                    nc.tensor.matmul(
                        pc[:],
                        lhsT=avg_row[:, blk * P : (blk + 1) * P],
                        rhs=ones_col[0:1, 0:1],
                        start=True,
                        stop=True,
                    )
                    nc.any.tensor_copy(out=pooled[:, blk : blk + 1], in_=pc[:])
                for blk in range(2):
                    pt = ps_small_pool.tile([P, P], F32, tag="ps")
                    nc.tensor.transpose(
                        pt[:], m[:, 0, blk * P : (blk + 1) * P], ident[:]
                    )
                    nc.vector.reduce_max(
                        out=pooled[:, 2 + blk : 3 + blk],
                        in_=pt[:],
                        axis=mybir.AxisListType.X,
                    )

                # --- shared MLP on both pooled vectors ---
                h_sb = small.tile([HID, 2], F32, tag="h")
                for j in range(2):  # 0: avg path, 1: max path
                    ph = ps_small_pool.tile([HID, 1], F32, tag="ps")
                    nc.tensor.matmul(
                        ph[:],
                        lhsT=w1_sb[:, 0, :],
                        rhs=pooled[:, 2 * j : 2 * j + 1],
                        start=True,
                        stop=False,
                    )
                    nc.tensor.matmul(
                        ph[:],
                        lhsT=w1_sb[:, 1, :],
                        rhs=pooled[:, 2 * j + 1 : 2 * j + 2],
                        start=False,
                        stop=True,
                    )
                    nc.scalar.activation(
                        out=h_sb[:, j : j + 1],
                        in_=ph[:],
                        func=AF.Relu,
                        bias=b1_sb[:],
                        scale=1.0,
                    )
                py = ps_small_pool.tile([1, C], F32, tag="ps")
                nc.tensor.matmul(
                    py[:], lhsT=h_sb[:, 0:1], rhs=w2_sb[:], start=True, stop=False
                )
                nc.tensor.matmul(
                    py[:], lhsT=h_sb[:, 1:2], rhs=w2_sb[:], start=False, stop=True
                )
                sig_row = small.tile([1, C], F32, tag="sig")
                nc.vector.tensor_add(out=sig_row[:], in0=py[:], in1=b2x2[:])
                nc.scalar.activation(out=sig_row[:], in_=sig_row[:], func=AF.Sigmoid)

                # broadcast scale row to all 128 partitions (outer product)
                pbc = ps_bc_pool.tile([P, C], F32)
                nc.tensor.matmul(
                    pbc[:], lhsT=ones_row[:], rhs=sig_row[:], start=True, stop=True
                )
                srep = srep_pool.tile([P, C], F32)
                nc.any.tensor_copy(out=srep[:], in_=pbc[:])
                srep3 = srep[:, :].rearrange("p (o c) -> p o c", o=1)

                # --- scale in place and store ---
                store_eng = {"scalar": nc.scalar, "sync": nc.sync,
                             "gpsimd": nc.gpsimd}[store_engine]
                for g in range(n_chunk):
                    ck = cks[g]
                    v3 = ck[:, :, :]
                    _, s_b = bass.broadcast_tensor_aps(v3, srep3)
                    nc.any.tensor_tensor(out=v3, in0=v3, in1=s_b, op=OP.mult)
                    store_eng.dma_start(
                        out=yv[img][:, g * chunk_t : (g + 1) * chunk_t, :], in_=ck[:]
                    )

    _split_multiwait(nc)
    return nc


# ---------------------------------------------------------------------------
# host-side driver
# ---------------------------------------------------------------------------

_CACHED = {}


def _get_nc():
    if "nc" not in _CACHED:
        _CACHED["nc"] = build_nc()
    return _CACHED["nc"]


def kernel(x, w1, b1, w2, b2):
    from concourse.bass_utils import run_bass_kernel_spmd

    x = np.ascontiguousarray(np.asarray(x), dtype=np.float32)
    assert x.shape == (B, 112, 112, C)
    xr = x.reshape(B, HW, C)
    ident = np.eye(P, dtype=np.float32)
    in_maps = []
    for c in range(N_CORES):
        shard = np.ascontiguousarray(
            xr[c * IMG_PER_CORE : (c + 1) * IMG_PER_CORE].reshape(
                IMG_PER_CORE * HW, C
            )
        )
        in_maps.append(
            {
                "x": shard,
                "w1": np.ascontiguousarray(w1, dtype=np.float32),
                "b1": np.ascontiguousarray(b1, dtype=np.float32),
                "w2": np.ascontiguousarray(w2, dtype=np.float32),
                "b2": np.ascontiguousarray(b2, dtype=np.float32),
                "ident": ident,
            }
        )
    nc = _get_nc()
    res = run_bass_kernel_spmd(nc, in_maps, list(range(N_CORES)))
    out = np.empty((B, HW, C), dtype=np.float32)
    for c in range(N_CORES):
        out[c * IMG_PER_CORE : (c + 1) * IMG_PER_CORE] = res.results[c]["y"].reshape(
            IMG_PER_CORE, HW, C
        )
    return out.reshape(B, 112, 112, C)


# revision 9
# speedup vs baseline: 1.0314x; 1.0255x over previous
"""ChannelAttention (CBAM-style) Trainium2 Bass kernel.

Reference computation (per batch image):
    avg = mean(x, spatial)             # [C]
    mx  = max(x, spatial)              # [C]
    s   = sigmoid(mlp(avg) + mlp(max)) # mlp: relu(p@w1+b1)@w2+b2
    y   = x * s[None, None, :]

Full shapes: x [32, 112, 112, 256] f32, w1 [256, 32], b1 [32], w2 [32, 256],
b2 [256].  Data-parallel over batch: each of the 8 NeuronCores handles 4
images; the tiny MLP weights are replicated.

Per-core layout: x viewed as [4, 12544, 256] rows.  Each image is split
into 128 partitions x 98 spatial rows ("(p t) c" with p=128, t=98), and
processed in 7 chunks of 14 spatial rows so DMA (fully contiguous 14336B
per partition per chunk) overlaps compute:
  - sum-pool: PE matmul with an all-ones stationary column accumulating
    [1, 256] channel sums in PSUM across all 98 row-tiles.
  - max-pool: ACT-engine tensor_tensor(max) accumulation into a
    [128, 7, 256] accumulator, folded 7->1 on DVE, then PE-transposed to
    channel-major and reduce_max'd to [128, 1] per 128-channel block.
  - MLP runs on PE/ACT at [K<=128, N<=256] sizes; sigmoid on ACT; the
    [1, 256] scale row is broadcast to [128, 256] via a K=1 outer-product
    matmul with ones.
  - scale: DVE tensor_tensor(mult) with a free-dim-broadcast AP, in place
    on the resident chunk, then stored back (ACT-issued HWDGE ring so
    loads on the SP ring proceed independently).
"""

import sys

import numpy as np

for _p in ("/opt/trn_rl_repo",):
    if _p not in sys.path:
        sys.path.append(_p)

import concourse.bass as bass
import concourse.tile as tile
from concourse import mybir

B, HW, C = 32, 112 * 112, 256  # 12544 spatial positions per image
HID = 32
N_CORES = 8
IMG_PER_CORE = B // N_CORES  # 4
P = 128
T = HW // P  # 98 spatial rows per partition
CHUNK_T = 14  # rows per chunk
N_CHUNK = T // CHUNK_T  # 7
MAXW = 7  # max-pool accumulator width (rows)
F32 = mybir.dt.float32


def _split_multiwait(nc, max_waits=1):
    """This walrus build rejects >1 sync wait per instruction.  Hoist
    excess waits onto InstNoOp instructions inserted just before, on the
    same engine (same semantics: the sequencer blocks on each in turn)."""
    for f in nc.m.functions:
        for bb in f.blocks:
            new_insts = []
            for ins in bb.instructions:
                si = ins.sync_info
                w = list(si.on_wait) if si and si.on_wait else []
                if len(w) > max_waits:
                    for j, ww in enumerate(w[:-max_waits]):
                        nop = mybir.InstNoOp(
                            name=f"{ins.name}.sw{j}",
                            engine=ins.engine,
                            sync_info=mybir.SyncInfo(on_wait=[ww], on_update=[]),
                        )
                        nc.register_instruction(nop, overwrite=True)
                        new_insts.append(nop)
                    si.on_wait = w[-max_waits:]
                new_insts.append(ins)
            bb.instructions = new_insts


def build_nc(n_img=IMG_PER_CORE, chunk_bufs=11, repeat=1, chunk_t=CHUNK_T,
             store_engine="scalar"):
    n_chunk = T // chunk_t
    assert chunk_t % MAXW == 0 or MAXW % chunk_t == 0
    nc = bass.Bass()
    rows = n_img * HW
    x_d = nc.declare_dram_parameter("x", [rows, C], F32, isOutput=False)
    w1_d = nc.declare_dram_parameter("w1", [C, HID], F32, isOutput=False)
    b1_d = nc.declare_dram_parameter("b1", [HID], F32, isOutput=False)
    w2_d = nc.declare_dram_parameter("w2", [HID, C], F32, isOutput=False)
    b2_d = nc.declare_dram_parameter("b2", [C], F32, isOutput=False)
    id_d = nc.declare_dram_parameter("ident", [P, P], F32, isOutput=False)
    y_d = nc.declare_dram_parameter("y", [rows, C], F32, isOutput=True)

    xv = x_d.rearrange("(i p t) c -> i p t c", i=n_img, p=P)
    yv = y_d.rearrange("(i p t) c -> i p t c", i=n_img, p=P)

    AF = mybir.ActivationFunctionType
    OP = mybir.AluOpType

    with tile.TileContext(nc) as tc:
        with (
            tc.tile_pool(name="singles", bufs=1) as singles,
            tc.tile_pool(name="chunks", bufs=chunk_bufs) as chunks_pool,
            tc.tile_pool(name="maxacc", bufs=2) as maxacc_pool,
            tc.tile_pool(name="srep", bufs=2) as srep_pool,
            tc.tile_pool(name="small", bufs=3) as small,
            tc.tile_pool(name="ps_sum", bufs=2, space="PSUM") as ps_sum_pool,
            tc.tile_pool(name="ps_small", bufs=2, space="PSUM") as ps_small_pool,
            tc.tile_pool(name="ps_bc", bufs=2, space="PSUM") as ps_bc_pool,
        ):
            # --- constants ---
            w1_sb = singles.tile([P, 2, HID], F32)
            nc.sync.dma_start(out=w1_sb[:], in_=w1_d.rearrange("(b p) h -> p b h", p=P))
            w2_sb = singles.tile([HID, C], F32)
            nc.sync.dma_start(out=w2_sb[:], in_=w2_d[:, :])
            b1_sb = singles.tile([HID, 1], F32)
            nc.sync.dma_start(out=b1_sb[:], in_=b1_d.rearrange("(p o) -> p o", o=1))
            b2x2 = singles.tile([1, C], F32)
            nc.sync.dma_start(out=b2x2[:], in_=b2_d.rearrange("(o c) -> o c", o=1))
            nc.scalar.mul(out=b2x2[:], in_=b2x2[:], mul=2.0)
            ident = singles.tile([P, P], F32)
            nc.sync.dma_start(out=ident[:], in_=id_d[:, :])
            ones_col = singles.tile([P, 1], F32)
            nc.vector.memset(ones_col[:], 1.0)
            ones_row = singles.tile([1, P], F32)
            nc.vector.memset(ones_row[:], 1.0)

            for img in [i for _ in range(repeat) for i in range(n_img)]:
                psum_sum = ps_sum_pool.tile([1, C], F32)
                w = min(MAXW, chunk_t)
                maxacc = maxacc_pool.tile([P, MAXW, C], F32)
                cks = []
                for g in range(n_chunk):
                    ck = chunks_pool.tile([P, chunk_t, C], F32, tag="ck")
                    cks.append(ck)
                    nc.sync.dma_start(
                        out=ck[:], in_=xv[img][:, g * chunk_t : (g + 1) * chunk_t, :]
                    )
                    # sum-pool: accumulate channel sums in PSUM
                    for t in range(chunk_t):
                        nc.tensor.matmul(
                            psum_sum[:],
                            lhsT=ones_col[:],
                            rhs=ck[:, t, :],
                            start=(g == 0 and t == 0),
                            stop=(g == n_chunk - 1 and t == chunk_t - 1),
                        )
                    # max-pool accumulate in MAXW-row slices (Tile routes the
                    # tensor_tensor ops between DVE and ACT)
                    for s in range(chunk_t // w):
                        sl = ck[:, s * w : (s + 1) * w, :]
                        if g == 0 and s == 0:
                            nc.any.tensor_copy(out=maxacc[:, 0:w, :], in_=sl)
                        else:
                            nc.any.tensor_tensor(
                                out=maxacc[:, 0:w, :],
                                in0=maxacc[:, 0:w, :],
                                in1=sl,
                                op=OP.max,
                            )

                # --- pooled vectors ---
                # avg: psum row -> sbuf row (scaled by 1/HW)
                avg_row = small.tile([1, C], F32, tag="avg_row")
                nc.scalar.activation(
                    out=avg_row[:], in_=psum_sum[:], func=AF.Copy, scale=1.0 / HW
                )
                # max: fold MAXW -> 1 in place on DVE
                m = maxacc
                nc.vector.tensor_tensor(m[:, 0, :], m[:, 0, :], m[:, 1, :], op=OP.max)
                nc.vector.tensor_tensor(m[:, 2, :], m[:, 2, :], m[:, 3, :], op=OP.max)
                nc.vector.tensor_tensor(m[:, 4, :], m[:, 4, :], m[:, 5, :], op=OP.max)
                nc.vector.tensor_tensor(m[:, 0, :], m[:, 0, :], m[:, 2, :], op=OP.max)
                nc.vector.tensor_tensor(m[:, 4, :], m[:, 4, :], m[:, 6, :], op=OP.max)
                nc.vector.tensor_tensor(m[:, 0, :], m[:, 0, :], m[:, 4, :], op=OP.max)

                # pooled columns: [128, 4] = avg_b0, avg_b1, max_b0, max_b1
                pooled = small.tile([P, 4], F32, tag="pooled")
                for blk in range(2):
                    pc = ps_small_pool.tile([P, 1], F32, tag="ps")
                    nc.tensor.matmul(
                        pc[:],
                        lhsT=avg_row[:, blk * P : (blk + 1) * P],
                        rhs=ones_col[0:1, 0:1],
                        start=True,
                        stop=True,
                    )
                    nc.any.tensor_copy(out=pooled[:, blk : blk + 1], in_=pc[:])
                for blk in range(2):
                    pt = ps_small_pool.tile([P, P], F32, tag="ps")
                    nc.tensor.transpose(
                        pt[:], m[:, 0, blk * P : (blk + 1) * P], ident[:]
                    )
                    nc.vector.reduce_max(
                        out=pooled[:, 2 + blk : 3 + blk],
                        in_=pt[:],
                        axis=mybir.AxisListType.X,
                    )

                # --- shared MLP on both pooled vectors ---
                h_sb = small.tile([HID, 2], F32, tag="h")
                for j in range(2):  # 0: avg path, 1: max path
                    ph = ps_small_pool.tile([HID, 1], F32, tag="ps")
                    nc.tensor.matmul(
                        ph[:],
                        lhsT=w1_sb[:, 0, :],
                        rhs=pooled[:, 2 * j : 2 * j + 1],
                        start=True,
                        stop=False,
                    )
                    nc.tensor.matmul(
                        ph[:],
                        lhsT=w1_sb[:, 1, :],
                        rhs=pooled[:, 2 * j + 1 : 2 * j + 2],
                        start=False,
                        stop=True,
                    )
                    nc.scalar.activation(
                        out=h_sb[:, j : j + 1],
                        in_=ph[:],
                        func=AF.Relu,
                        bias=b1_sb[:],
                        scale=1.0,
                    )
                py = ps_small_pool.tile([1, C], F32, tag="ps")
                nc.tensor.matmul(
                    py[:], lhsT=h_sb[:, 0:1], rhs=w2_sb[:], start=True, stop=False
                )
                nc.tensor.matmul(
                    py[:], lhsT=h_sb[:, 1:2], rhs=w2_sb[:], start=False, stop=True
                )
                sig_row = small.tile([1, C], F32, tag="sig")
                nc.vector.tensor_add(out=sig_row[:], in0=py[:], in1=b2x2[:])
                nc.scalar.activation(out=sig_row[:], in_=sig_row[:], func=AF.Sigmoid)

                # broadcast scale row to all 128 partitions (outer product)
                pbc = ps_bc_pool.tile([P, C], F32)
                nc.tensor.matmul(
                    pbc[:], lhsT=ones_row[:], rhs=sig_row[:], start=True, stop=True
                )
                srep = srep_pool.tile([P, C], F32)
                nc.any.tensor_copy(out=srep[:], in_=pbc[:])
                srep3 = srep[:, :].rearrange("p (o c) -> p o c", o=1)

                # --- scale in place and store ---
                store_eng = {"scalar": nc.scalar, "sync": nc.sync,
                             "gpsimd": nc.gpsimd}[store_engine]
                for g in range(n_chunk):
                    ck = cks[g]
                    v3 = ck[:, :, :]
                    _, s_b = bass.broadcast_tensor_aps(v3, srep3)
                    nc.any.tensor_tensor(out=v3, in0=v3, in1=s_b, op=OP.mult)
                    store_eng.dma_start(
                        out=yv[img][:, g * chunk_t : (g + 1) * chunk_t, :], in_=ck[:]
                    )

    _split_multiwait(nc)
    return nc


# ---------------------------------------------------------------------------
# host-side driver
# ---------------------------------------------------------------------------

_CACHED = {}


def _get_nc():
    if "nc" not in _CACHED:
        _CACHED["nc"] = build_nc()
    return _CACHED["nc"]


def kernel(x, w1, b1, w2, b2):
    from concourse.bass_utils import run_bass_kernel_spmd

    x = np.ascontiguousarray(np.asarray(x), dtype=np.float32)
    assert x.shape == (B, 112, 112, C)
    xr = x.reshape(B, HW, C)
    ident = np.eye(P, dtype=np.float32)
    in_maps = []
    for c in range(N_CORES):
        shard = np.ascontiguousarray(
            xr[c * IMG_PER_CORE : (c + 1) * IMG_PER_CORE].reshape(
                IMG_PER_CORE * HW, C
            )
        )
        in_maps.append(
            {
                "x": shard,
                "w1": np.ascontiguousarray(w1, dtype=np.float32),
                "b1": np.ascontiguousarray(b1, dtype=np.float32),
                "w2": np.ascontiguousarray(w2, dtype=np.float32),
                "b2": np.ascontiguousarray(b2, dtype=np.float32),
                "ident": ident,
            }
        )
    nc = _get_nc()
    res = run_bass_kernel_spmd(nc, in_maps, list(range(N_CORES)))
    out = np.empty((B, HW, C), dtype=np.float32)
    for c in range(N_CORES):
        out[c * IMG_PER_CORE : (c + 1) * IMG_PER_CORE] = res.results[c]["y"].reshape(
            IMG_PER_CORE, HW, C
        )
    return out.reshape(B, 112, 112, C)


# revision 10
# speedup vs baseline: 1.0338x; 1.0023x over previous
"""ChannelAttention (CBAM-style) Trainium2 Bass kernel.

Reference computation (per batch image):
    avg = mean(x, spatial)             # [C]
    mx  = max(x, spatial)              # [C]
    s   = sigmoid(mlp(avg) + mlp(max)) # mlp: relu(p@w1+b1)@w2+b2
    y   = x * s[None, None, :]

Full shapes: x [32, 112, 112, 256] f32, w1 [256, 32], b1 [32], w2 [32, 256],
b2 [256].  Data-parallel over batch: each of the 8 NeuronCores handles 4
images; the tiny MLP weights are replicated.

Per-core layout: x viewed as [4, 12544, 256] rows.  Each image is split
into 128 partitions x 98 spatial rows ("(p t) c" with p=128, t=98), and
processed in 7 chunks of 14 spatial rows so DMA (fully contiguous 14336B
per partition per chunk) overlaps compute:
  - sum-pool: PE matmul with an all-ones stationary column accumulating
    [1, 256] channel sums in PSUM across all 98 row-tiles.
  - max-pool: ACT-engine tensor_tensor(max) accumulation into a
    [128, 7, 256] accumulator, folded 7->1 on DVE, then PE-transposed to
    channel-major and reduce_max'd to [128, 1] per 128-channel block.
  - MLP runs on PE/ACT at [K<=128, N<=256] sizes; sigmoid on ACT; the
    [1, 256] scale row is broadcast to [128, 256] via a K=1 outer-product
    matmul with ones.
  - scale: DVE tensor_tensor(mult) with a free-dim-broadcast AP, in place
    on the resident chunk, then stored back (ACT-issued HWDGE ring so
    loads on the SP ring proceed independently).
"""

import sys

import numpy as np

for _p in ("/opt/trn_rl_repo",):
    if _p not in sys.path:
        sys.path.append(_p)

import concourse.bass as bass
import concourse.tile as tile
from concourse import mybir

B, HW, C = 32, 112 * 112, 256  # 12544 spatial positions per image
HID = 32
N_CORES = 8
IMG_PER_CORE = B // N_CORES  # 4
P = 128
T = HW // P  # 98 spatial rows per partition
CHUNK_T = 14  # rows per chunk
N_CHUNK = T // CHUNK_T  # 7
MAXW = 7  # max-pool accumulator width (rows)
F32 = mybir.dt.float32


def _split_multiwait(nc, max_waits=1):
    """This walrus build rejects >1 sync wait per instruction.  Hoist
    excess waits onto InstNoOp instructions inserted just before, on the
    same engine (same semantics: the sequencer blocks on each in turn)."""
    for f in nc.m.functions:
        for bb in f.blocks:
            new_insts = []
            for ins in bb.instructions:
                si = ins.sync_info
                w = list(si.on_wait) if si and si.on_wait else []
                if len(w) > max_waits:
                    for j, ww in enumerate(w[:-max_waits]):
                        nop = mybir.InstNoOp(
                            name=f"{ins.name}.sw{j}",
                            engine=ins.engine,
                            sync_info=mybir.SyncInfo(on_wait=[ww], on_update=[]),
                        )
                        nc.register_instruction(nop, overwrite=True)
                        new_insts.append(nop)
                    si.on_wait = w[-max_waits:]
                new_insts.append(ins)
            bb.instructions = new_insts


def build_nc(n_img=IMG_PER_CORE, chunk_bufs=10, repeat=1, chunk_t=CHUNK_T,
             store_engine="scalar"):
    n_chunk = T // chunk_t
    assert chunk_t % MAXW == 0 or MAXW % chunk_t == 0
    nc = bass.Bass()
    rows = n_img * HW
    x_d = nc.declare_dram_parameter("x", [rows, C], F32, isOutput=False)
    w1_d = nc.declare_dram_parameter("w1", [C, HID], F32, isOutput=False)
    b1_d = nc.declare_dram_parameter("b1", [HID], F32, isOutput=False)
    w2_d = nc.declare_dram_parameter("w2", [HID, C], F32, isOutput=False)
    b2_d = nc.declare_dram_parameter("b2", [C], F32, isOutput=False)
    id_d = nc.declare_dram_parameter("ident", [P, P], F32, isOutput=False)
    y_d = nc.declare_dram_parameter("y", [rows, C], F32, isOutput=True)

    xv = x_d.rearrange("(i p t) c -> i p t c", i=n_img, p=P)
    yv = y_d.rearrange("(i p t) c -> i p t c", i=n_img, p=P)

    AF = mybir.ActivationFunctionType
    OP = mybir.AluOpType

    with tile.TileContext(nc) as tc:
        with (
            tc.tile_pool(name="singles", bufs=1) as singles,
            tc.tile_pool(name="chunks", bufs=chunk_bufs) as chunks_pool,
            tc.tile_pool(name="maxacc", bufs=4) as maxacc_pool,
            tc.tile_pool(name="srep", bufs=2) as srep_pool,
            tc.tile_pool(name="small", bufs=3) as small,
            tc.tile_pool(name="ps_sum", bufs=2, space="PSUM") as ps_sum_pool,
            tc.tile_pool(name="ps_small", bufs=2, space="PSUM") as ps_small_pool,
            tc.tile_pool(name="ps_bc", bufs=2, space="PSUM") as ps_bc_pool,
        ):
            # --- constants ---
            w1_sb = singles.tile([P, 2, HID], F32)
            nc.sync.dma_start(out=w1_sb[:], in_=w1_d.rearrange("(b p) h -> p b h", p=P))
            w2_sb = singles.tile([HID, C], F32)
            nc.sync.dma_start(out=w2_sb[:], in_=w2_d[:, :])
            b1_sb = singles.tile([HID, 1], F32)
            nc.sync.dma_start(out=b1_sb[:], in_=b1_d.rearrange("(p o) -> p o", o=1))
            b2x2 = singles.tile([1, C], F32)
            nc.sync.dma_start(out=b2x2[:], in_=b2_d.rearrange("(o c) -> o c", o=1))
            nc.scalar.mul(out=b2x2[:], in_=b2x2[:], mul=2.0)
            ident = singles.tile([P, P], F32)
            nc.sync.dma_start(out=ident[:], in_=id_d[:, :])
            ones_col = singles.tile([P, 1], F32)
            nc.vector.memset(ones_col[:], 1.0)
            ones_row = singles.tile([1, P], F32)
            nc.vector.memset(ones_row[:], 1.0)

            for img in [i for _ in range(repeat) for i in range(n_img)]:
                psum_sum = ps_sum_pool.tile([1, C], F32)
                w = min(MAXW, chunk_t)
                # two independent accumulators so the per-chunk max ops form
                # two parallel chains (one lands on DVE, one on ACT)
                maxacc = maxacc_pool.tile([P, MAXW, C], F32, tag="ma")
                maxaccB = maxacc_pool.tile([P, MAXW, C], F32, tag="ma")
                cks = []
                for g in range(n_chunk):
                    ck = chunks_pool.tile([P, chunk_t, C], F32, tag="ck")
                    cks.append(ck)
                    nc.sync.dma_start(
                        out=ck[:], in_=xv[img][:, g * chunk_t : (g + 1) * chunk_t, :]
                    )
                    # sum-pool: accumulate channel sums in PSUM
                    for t in range(chunk_t):
                        nc.tensor.matmul(
                            psum_sum[:],
                            lhsT=ones_col[:],
                            rhs=ck[:, t, :],
                            start=(g == 0 and t == 0),
                            stop=(g == n_chunk - 1 and t == chunk_t - 1),
                        )
                    # max-pool accumulate in MAXW-row slices (Tile routes the
                    # tensor_tensor ops between DVE and ACT)
                    for s in range(chunk_t // w):
                        sl = ck[:, s * w : (s + 1) * w, :]
                        acc = maxacc if s % 2 == 0 else maxaccB
                        if g == 0 and s < 2:
                            nc.any.tensor_copy(out=acc[:, 0:w, :], in_=sl)
                        else:
                            nc.any.tensor_tensor(
                                out=acc[:, 0:w, :],
                                in0=acc[:, 0:w, :],
                                in1=sl,
                                op=OP.max,
                            )

                # --- pooled vectors ---
                # avg: psum row -> sbuf row (scaled by 1/HW)
                avg_row = small.tile([1, C], F32, tag="avg_row")
                nc.scalar.activation(
                    out=avg_row[:], in_=psum_sum[:], func=AF.Copy, scale=1.0 / HW
                )
                # max: fold the two accumulators, tree-wise, in place
                m, mb = maxacc, maxaccB
                nc.any.tensor_tensor(m[:, 0:3, :], m[:, 0:3, :], m[:, 3:6, :], op=OP.max)
                nc.any.tensor_tensor(mb[:, 0:3, :], mb[:, 0:3, :], mb[:, 3:6, :], op=OP.max)
                nc.any.tensor_tensor(m[:, 0:1, :], m[:, 0:1, :], m[:, 6:7, :], op=OP.max)
                nc.any.tensor_tensor(mb[:, 0:1, :], mb[:, 0:1, :], mb[:, 6:7, :], op=OP.max)
                nc.any.tensor_tensor(m[:, 0:1, :], m[:, 0:1, :], m[:, 1:2, :], op=OP.max)
                nc.any.tensor_tensor(mb[:, 0:1, :], mb[:, 0:1, :], mb[:, 1:2, :], op=OP.max)
                nc.any.tensor_tensor(m[:, 0:1, :], m[:, 0:1, :], m[:, 2:3, :], op=OP.max)
                nc.any.tensor_tensor(mb[:, 0:1, :], mb[:, 0:1, :], mb[:, 2:3, :], op=OP.max)
                nc.any.tensor_tensor(m[:, 0:1, :], m[:, 0:1, :], mb[:, 0:1, :], op=OP.max)

                # pooled columns: [128, 4] = avg_b0, avg_b1, max_b0, max_b1
                pooled = small.tile([P, 4], F32, tag="pooled")
                for blk in range(2):
                    pc = ps_small_pool.tile([P, 1], F32, tag="ps")
                    nc.tensor.matmul(
                        pc[:],
                        lhsT=avg_row[:, blk * P : (blk + 1) * P],
                        rhs=ones_col[0:1, 0:1],
                        start=True,
                        stop=True,
                    )
                    nc.any.tensor_copy(out=pooled[:, blk : blk + 1], in_=pc[:])
                for blk in range(2):
                    pt = ps_small_pool.tile([P, P], F32, tag="ps")
                    nc.tensor.transpose(
                        pt[:], m[:, 0, blk * P : (blk + 1) * P], ident[:]
                    )
                    nc.vector.reduce_max(
                        out=pooled[:, 2 + blk : 3 + blk],
                        in_=pt[:],
                        axis=mybir.AxisListType.X,
                    )

                # --- shared MLP on both pooled vectors ---
                h_sb = small.tile([HID, 2], F32, tag="h")
                for j in range(2):  # 0: avg path, 1: max path
                    ph = ps_small_pool.tile([HID, 1], F32, tag="ps")
                    nc.tensor.matmul(
                        ph[:],
                        lhsT=w1_sb[:, 0, :],
                        rhs=pooled[:, 2 * j : 2 * j + 1],
                        start=True,
                        stop=False,
                    )
                    nc.tensor.matmul(
                        ph[:],
                        lhsT=w1_sb[:, 1, :],
                        rhs=pooled[:, 2 * j + 1 : 2 * j + 2],
                        start=False,
                        stop=True,
                    )
                    nc.scalar.activation(
                        out=h_sb[:, j : j + 1],
                        in_=ph[:],
                        func=AF.Relu,
                        bias=b1_sb[:],
                        scale=1.0,
                    )
                py = ps_small_pool.tile([1, C], F32, tag="ps")
                nc.tensor.matmul(
                    py[:], lhsT=h_sb[:, 0:1], rhs=w2_sb[:], start=True, stop=False
                )
                nc.tensor.matmul(
                    py[:], lhsT=h_sb[:, 1:2], rhs=w2_sb[:], start=False, stop=True
                )
                sig_row = small.tile([1, C], F32, tag="sig")
                nc.vector.tensor_add(out=sig_row[:], in0=py[:], in1=b2x2[:])
                nc.scalar.activation(out=sig_row[:], in_=sig_row[:], func=AF.Sigmoid)

                # broadcast scale row to all 128 partitions (outer product)
                pbc = ps_bc_pool.tile([P, C], F32)
                nc.tensor.matmul(
                    pbc[:], lhsT=ones_row[:], rhs=sig_row[:], start=True, stop=True
                )
                srep = srep_pool.tile([P, C], F32)
                nc.any.tensor_copy(out=srep[:], in_=pbc[:])
                srep3 = srep[:, :].rearrange("p (o c) -> p o c", o=1)

                # --- scale in place and store ---
                store_eng = {"scalar": nc.scalar, "sync": nc.sync,
                             "gpsimd": nc.gpsimd}[store_engine]
                for g in range(n_chunk):
                    ck = cks[g]
                    for h in range(chunk_t // w):
                        v3 = ck[:, h * w : (h + 1) * w, :]
                        _, s_b = bass.broadcast_tensor_aps(v3, srep3)
                        nc.any.tensor_tensor(out=v3, in0=v3, in1=s_b, op=OP.mult)
                    store_eng.dma_start(
                        out=yv[img][:, g * chunk_t : (g + 1) * chunk_t, :], in_=ck[:]
                    )

    _split_multiwait(nc)
    return nc


# ---------------------------------------------------------------------------
# host-side driver
# ---------------------------------------------------------------------------

_CACHED = {}


def _get_nc():
    if "nc" not in _CACHED:
        _CACHED["nc"] = build_nc()
    return _CACHED["nc"]


def kernel(x, w1, b1, w2, b2):
    from concourse.bass_utils import run_bass_kernel_spmd

    x = np.ascontiguousarray(np.asarray(x), dtype=np.float32)
    assert x.shape == (B, 112, 112, C)
    xr = x.reshape(B, HW, C)
    ident = np.eye(P, dtype=np.float32)
    in_maps = []
    for c in range(N_CORES):
        shard = np.ascontiguousarray(
            xr[c * IMG_PER_CORE : (c + 1) * IMG_PER_CORE].reshape(
                IMG_PER_CORE * HW, C
            )
        )
        in_maps.append(
            {
                "x": shard,
                "w1": np.ascontiguousarray(w1, dtype=np.float32),
                "b1": np.ascontiguousarray(b1, dtype=np.float32),
                "w2": np.ascontiguousarray(w2, dtype=np.float32),
                "b2": np.ascontiguousarray(b2, dtype=np.float32),
                "ident": ident,
            }
        )
    nc = _get_nc()
    res = run_bass_kernel_spmd(nc, in_maps, list(range(N_CORES)))
    out = np.empty((B, HW, C), dtype=np.float32)
    for c in range(N_CORES):
        out[c * IMG_PER_CORE : (c + 1) * IMG_PER_CORE] = res.results[c]["y"].reshape(
            IMG_PER_CORE, HW, C
        )
    return out.reshape(B, 112, 112, C)
